# revision 1
# baseline (speedup 1.0000x reference)
"""DGCNN prediction head on 8 Trainium2 NeuronCores.

Data-parallel over batch B=8: each core runs the full pipeline for one
sample (C=64 channels, N=4096 points, k=20 neighbors).

Per-core pipeline (all on one NeuronCore, no collectives):
  1. pairwise ranking R[i,j] = 2<x_i,x_j> - ||x_j||^2 via PE matmul with an
     augmented contract row (row 64 of lhsT = -1, row 64 of rhs = ||x_j||^2).
     (-||x_i||^2 is a per-row constant and cannot change the top-k order.)
  2. exact top-20 per row with DVE max8/max_index/match_replace (3 rounds).
  3. EdgeConv1 is linear before the LReLU, so it is precomputed per point:
       conv1(i,j) = Wn x_j + (Wc - Wn) x_i  with BN1 folded in
     A' = s1*(Wn x)        -> transposed to DRAM table, row-gathered by index
     B' = s1*((Wc-Wn) x)+t1-> kept on-chip, broadcast-added per query block
  4. e1 = lrelu(A'_j + B'_i) per edge; PE-transpose to channel-major;
     EdgeConv2 as 64x64 matmul (BN2 scale folded into W2, bias t2 added
     during the PSUM drain); max over k on GPSIMD; lrelu (monotone ops
     commute with max since s2 >= 0).
  5. point MLP 64->256->128->1 with BN scales folded into weights, biases
     added during PSUM drains, lrelu on GPSIMD.
"""

import numpy as np

C = 64
K = 20
NEG = 0.2
EPS = 1e-5
NCORES = 8
N_FULL = 4096
NEG_FILL = -3.0e38

_cache = {}


def build_nc(n):
    from contextlib import ExitStack

    import concourse.bass as bass
    import concourse.bacc as bacc
    import concourse.mybir as mybir
    import concourse.tile as tile
    from concourse.masks import make_identity

    f32 = mybir.dt.float32
    u32 = mybir.dt.uint32
    AF = mybir.ActivationFunctionType
    OP = mybir.AluOpType

    nblk = n // 128
    nchk = n // 512

    nc = bacc.Bacc("TRN2", target_bir_lowering=False, debug=False,
                   num_devices=NCORES)

    x_d = nc.dram_tensor("x", [C, n], f32, kind="ExternalInput")
    wnT_d = nc.dram_tensor("wnT", [C, C], f32, kind="ExternalInput")
    wcnT_d = nc.dram_tensor("wcnT", [C, C], f32, kind="ExternalInput")
    t1_d = nc.dram_tensor("t1", [C, 1], f32, kind="ExternalInput")
    w2T_d = nc.dram_tensor("w2T", [C, C], f32, kind="ExternalInput")
    t2_d = nc.dram_tensor("t2", [C, 1], f32, kind="ExternalInput")
    w1aT_d = nc.dram_tensor("w1aT", [C, 128], f32, kind="ExternalInput")
    w1bT_d = nc.dram_tensor("w1bT", [C, 128], f32, kind="ExternalInput")
    tm1a_d = nc.dram_tensor("tm1a", [128, 1], f32, kind="ExternalInput")
    tm1b_d = nc.dram_tensor("tm1b", [128, 1], f32, kind="ExternalInput")
    w2maT_d = nc.dram_tensor("w2maT", [128, 128], f32, kind="ExternalInput")
    w2mbT_d = nc.dram_tensor("w2mbT", [128, 128], f32, kind="ExternalInput")
    tm2_d = nc.dram_tensor("tm2", [128, 1], f32, kind="ExternalInput")
    w3T_d = nc.dram_tensor("w3T", [128, 1], f32, kind="ExternalInput")
    b3_d = nc.dram_tensor("b3", [1, 1], f32, kind="ExternalInput")
    out_d = nc.dram_tensor("out", [1, n], f32, kind="ExternalOutput")

    with tile.TileContext(nc) as tc, ExitStack() as top:
        cpool = top.enter_context(tc.tile_pool(name="consts", bufs=1))
        dpool = top.enter_context(tc.tile_pool(name="dram", bufs=1, space="DRAM"))
        xpool = top.enter_context(tc.tile_pool(name="xaug", bufs=1))
        hpool = top.enter_context(tc.tile_pool(name="hout", bufs=1))

        # --- constants / weights ---
        ident = cpool.tile([128, 128], f32, tag="ident")
        make_identity(nc, ident[:])
        ones64 = cpool.tile([C, 1], f32, tag="ones64")
        nc.vector.memset(ones64[:], 1.0)

        def load_const(dram, shape, tag):
            t = cpool.tile(shape, f32, tag=tag)
            nc.sync.dma_start(t[:], dram[:])
            return t

        wnT = load_const(wnT_d, [C, C], "wnT")
        wcnT = load_const(wcnT_d, [C, C], "wcnT")
        t1 = load_const(t1_d, [C, 1], "t1")
        w2T = load_const(w2T_d, [C, C], "w2T")
        t2 = load_const(t2_d, [C, 1], "t2")
        w1aT = load_const(w1aT_d, [C, 128], "w1aT")
        w1bT = load_const(w1bT_d, [C, 128], "w1bT")
        tm1a = load_const(tm1a_d, [128, 1], "tm1a")
        tm1b = load_const(tm1b_d, [128, 1], "tm1b")
        w2maT = load_const(w2maT_d, [128, 128], "w2maT")
        w2mbT = load_const(w2mbT_d, [128, 128], "w2mbT")
        tm2 = load_const(tm2_d, [128, 1], "tm2")
        w3T = load_const(w3T_d, [128, 1], "w3T")
        b3 = load_const(b3_d, [1, 1], "b3")

        At = dpool.tile([n, C], f32, tag="At")          # A' transposed table
        xaug = xpool.tile([C + 1, n], f32, tag="xaug")   # rows 0..63 = x, row 64 = ||x_j||^2
        x2aug = xpool.tile([C + 1, n], f32, tag="x2aug") # rows 0..63 = 2x, row 64 = -1
        Bt = xpool.tile([128, C * nblk], f32, tag="Bt")  # B' transposed, block j at cols 64j
        H = hpool.tile([C, n], f32, tag="H")             # per-point features after edge max
        osb = hpool.tile([1, n], f32, tag="osb")

        # ---------------- stage 0: tables ----------------
        with tc.tile_pool(name="s0sb", bufs=2) as s0sb, \
             tc.tile_pool(name="s0ps", bufs=3, space="PSUM") as s0ps:
            nc.sync.dma_start(xaug[:C, :], x_d[:])
            nc.scalar.activation(out=x2aug[:C, :], in_=xaug[:C, :],
                                 func=AF.Copy, scale=2.0)
            nc.vector.memset(x2aug[C:C + 1, :], -1.0)
            for ch in range(nchk):
                cs = slice(512 * ch, 512 * (ch + 1))
                xsq = s0sb.tile([C, 512], f32, tag="xsq")
                nc.scalar.activation(out=xsq[:], in_=xaug[:C, cs], func=AF.Square)
                psxx = s0ps.tile([1, 512], f32, tag="s0p", space="PSUM")
                nc.tensor.matmul(out=psxx[:], lhsT=ones64[:], rhs=xsq[:],
                                 start=True, stop=True)
                nc.scalar.copy(out=xaug[C:C + 1, cs], in_=psxx[:])
            for ch in range(nchk):
                cs = slice(512 * ch, 512 * (ch + 1))
                psa = s0ps.tile([C, 512], f32, tag="s0p", space="PSUM")
                nc.tensor.matmul(out=psa[:], lhsT=wnT[:], rhs=xaug[:C, cs],
                                 start=True, stop=True)
                ap = s0sb.tile([C, 512], f32, tag="ap")
                nc.scalar.copy(out=ap[:], in_=psa[:])
                psb = s0ps.tile([C, 512], f32, tag="s0p", space="PSUM")
                nc.tensor.matmul(out=psb[:], lhsT=wcnT[:], rhs=xaug[:C, cs],
                                 start=True, stop=True)
                bp = s0sb.tile([C, 512], f32, tag="bp")
                nc.scalar.activation(out=bp[:], in_=psb[:], func=AF.Identity,
                                     bias=t1[:], scale=1.0)
                for j in range(4):
                    blk = 4 * ch + j
                    js = slice(128 * j, 128 * (j + 1))
                    pta = s0ps.tile([128, C], f32, tag="s0p", space="PSUM")
                    nc.tensor.transpose(out=pta[:], in_=ap[:, js],
                                        identity=ident[:C, :C])
                    ast = s0sb.tile([128, C], f32, tag="ast")
                    nc.scalar.copy(out=ast[:], in_=pta[:])
                    nc.sync.dma_start(At[128 * blk:128 * (blk + 1), :], ast[:])
                    ptb = s0ps.tile([128, C], f32, tag="s0p", space="PSUM")
                    nc.tensor.transpose(out=ptb[:], in_=bp[:, js],
                                        identity=ident[:C, :C])
                    nc.scalar.copy(out=Bt[:, C * blk:C * (blk + 1)], in_=ptb[:])

        # ---------------- stage 1: blocks ----------------
        with tc.tile_pool(name="rpool", bufs=2) as rpool, \
             tc.tile_pool(name="vpool", bufs=8) as vpool, \
             tc.tile_pool(name="gpool", bufs=2) as gpool, \
             tc.tile_pool(name="epool", bufs=2) as epool, \
             tc.tile_pool(name="wpool", bufs=2) as wpool, \
             tc.tile_pool(name="tpool", bufs=2) as tpool, \
             tc.tile_pool(name="psR", bufs=2, space="PSUM") as psR, \
             tc.tile_pool(name="psT", bufs=2, space="PSUM") as psT, \
             tc.tile_pool(name="psE", bufs=2, space="PSUM") as psE:

            r_tiles = {}

            def emit_pairwise(b):
                R0 = rpool.tile([128, n], f32, tag="R")
                bs = slice(128 * b, 128 * (b + 1))
                for ch in range(nchk):
                    cs = slice(512 * ch, 512 * (ch + 1))
                    ps = psR.tile([128, 512], f32, tag="psr", space="PSUM")
                    nc.tensor.matmul(out=ps[:], lhsT=x2aug[:, bs],
                                     rhs=xaug[:, cs], start=True, stop=True)
                    nc.scalar.copy(out=R0[:, cs], in_=ps[:])
                r_tiles[b] = R0

            def emit_edge(b):
                R0 = r_tiles.pop(b)
                bs = slice(128 * b, 128 * (b + 1))
                v1 = vpool.tile([128, 8], f32, tag="v1")
                v2 = vpool.tile([128, 8], f32, tag="v2")
                v3 = vpool.tile([128, 8], f32, tag="v3")
                i1 = vpool.tile([128, 8], u32, tag="i1")
                i2 = vpool.tile([128, 8], u32, tag="i2")
                i3 = vpool.tile([128, 8], u32, tag="i3")
                nc.vector.max(out=v1[:], in_=R0[:])
                nc.vector.max_index(out=i1[:], in_max=v1[:], in_values=R0[:])
                nc.vector.match_replace(out=R0[:], in_to_replace=v1[:],
                                        in_values=R0[:], imm_value=NEG_FILL)
                nc.vector.max(out=v2[:], in_=R0[:])
                nc.vector.max_index(out=i2[:], in_max=v2[:], in_values=R0[:])
                nc.vector.match_replace(out=R0[:], in_to_replace=v2[:],
                                        in_values=R0[:], imm_value=NEG_FILL)
                nc.vector.max(out=v3[:], in_=R0[:])
                nc.vector.max_index(out=i3[:], in_max=v3[:], in_values=R0[:])

                G = gpool.tile([128, K * C], f32, tag="G")
                isrc = [i1] * 8 + [i2] * 8 + [i3] * 4
                for k in range(K):
                    col = k % 8
                    nc.gpsimd.indirect_dma_start(
                        out=G[:, C * k:C * (k + 1)], out_offset=None,
                        in_=At[:],
                        in_offset=bass.IndirectOffsetOnAxis(
                            ap=isrc[k][:, col:col + 1], axis=0))

                # e1 = lrelu(G + B'_i)
                bb = Bt[:, C * b:C * (b + 1)].rearrange(
                    "p (k c) -> p k c", k=1).to_broadcast([128, K, C])
                nc.vector.tensor_tensor(
                    out=G[:].rearrange("p (k c) -> p k c", k=K),
                    in0=G[:].rearrange("p (k c) -> p k c", k=K),
                    in1=bb, op=OP.add)
                nc.vector.scalar_tensor_tensor(
                    out=G[:], in0=G[:], scalar=NEG, in1=G[:],
                    op0=OP.mult, op1=OP.max)

                # transpose to channel-major: 20 PE transposes [128,64]->[64,128]
                e1T = gpool.tile([C, K * 128], f32, tag="e1T")
                for grp in range(5):
                    pt = psT.tile([C, 512], f32, tag="pst", space="PSUM")
                    for s in range(4):
                        k = 4 * grp + s
                        nc.tensor.transpose(
                            out=pt[:, 128 * s:128 * (s + 1)],
                            in_=G[:, C * k:C * (k + 1)],
                            identity=ident[:])
                    nc.scalar.copy(out=e1T[:, 512 * grp:512 * (grp + 1)],
                                   in_=pt[:])

                # conv2 (w_k2 with bn2 scale folded), t2 added in drain
                ew = wpool.tile([C, K * 128], f32, tag="ew")
                for grp in range(5):
                    pe = psE.tile([C, 512], f32, tag="pse", space="PSUM")
                    for s in range(4):
                        k = 4 * grp + s
                        nc.tensor.matmul(
                            out=pe[:, 128 * s:128 * (s + 1)],
                            lhsT=w2T[:],
                            rhs=e1T[:, 128 * k:128 * (k + 1)],
                            start=True, stop=True)
                    nc.scalar.activation(
                        out=ew[:, 512 * grp:512 * (grp + 1)], in_=pe[:],
                        func=AF.Identity, bias=t2[:], scale=1.0)

                # max over k (GPSIMD tree), then lrelu -> H
                m1 = tpool.tile([C, 10 * 128], f32, tag="m1")
                nc.vector.tensor_tensor(out=m1[:], in0=ew[:, :1280],
                                        in1=ew[:, 1280:], op=OP.max)
                m2 = tpool.tile([C, 5 * 128], f32, tag="m2")
                nc.vector.tensor_tensor(out=m2[:], in0=m1[:, :640],
                                        in1=m1[:, 640:], op=OP.max)
                m3 = tpool.tile([C, 2 * 128], f32, tag="m3")
                nc.vector.tensor_tensor(out=m3[:], in0=m2[:, :256],
                                        in1=m2[:, 256:512], op=OP.max)
                m4 = tpool.tile([C, 128], f32, tag="m4")
                nc.vector.tensor_tensor(out=m4[:], in0=m3[:, :128],
                                        in1=m3[:, 128:], op=OP.max)
                nc.vector.tensor_tensor(out=m4[:], in0=m4[:],
                                        in1=m2[:, 512:], op=OP.max)
                nc.vector.scalar_tensor_tensor(
                    out=H[:, bs], in0=m4[:], scalar=NEG, in1=m4[:],
                    op0=OP.mult, op1=OP.max)

            emit_pairwise(0)
            for b in range(nblk):
                if b + 1 < nblk:
                    emit_pairwise(b + 1)
                emit_edge(b)

        # ---------------- stage 2: point MLP ----------------
        with tc.tile_pool(name="mlpsb", bufs=2) as mlpsb, \
             tc.tile_pool(name="mlpps", bufs=4, space="PSUM") as mlpps:
            for ch in range(nchk):
                cs = slice(512 * ch, 512 * (ch + 1))
                l1a = mlpsb.tile([128, 512], f32, tag="l1a")
                l1b = mlpsb.tile([128, 512], f32, tag="l1b")
                ps1a = mlpps.tile([128, 512], f32, tag="mlpp", space="PSUM")
                nc.tensor.matmul(out=ps1a[:], lhsT=w1aT[:], rhs=H[:, cs],
                                 start=True, stop=True)
                nc.scalar.activation(out=l1a[:], in_=ps1a[:],
                                     func=AF.Identity, bias=tm1a[:], scale=1.0)
                nc.vector.scalar_tensor_tensor(
                    out=l1a[:], in0=l1a[:], scalar=NEG, in1=l1a[:],
                    op0=OP.mult, op1=OP.max)
                ps1b = mlpps.tile([128, 512], f32, tag="mlpp", space="PSUM")
                nc.tensor.matmul(out=ps1b[:], lhsT=w1bT[:], rhs=H[:, cs],
                                 start=True, stop=True)
                nc.scalar.activation(out=l1b[:], in_=ps1b[:],
                                     func=AF.Identity, bias=tm1b[:], scale=1.0)
                nc.vector.scalar_tensor_tensor(
                    out=l1b[:], in0=l1b[:], scalar=NEG, in1=l1b[:],
                    op0=OP.mult, op1=OP.max)
                ps2 = mlpps.tile([128, 512], f32, tag="mlpp", space="PSUM")
                nc.tensor.matmul(out=ps2[:], lhsT=w2maT[:], rhs=l1a[:],
                                 start=True, stop=False)
                nc.tensor.matmul(out=ps2[:], lhsT=w2mbT[:], rhs=l1b[:],
                                 start=False, stop=True)
                l2 = mlpsb.tile([128, 512], f32, tag="l2")
                nc.scalar.activation(out=l2[:], in_=ps2[:],
                                     func=AF.Identity, bias=tm2[:], scale=1.0)
                nc.vector.scalar_tensor_tensor(
                    out=l2[:], in0=l2[:], scalar=NEG, in1=l2[:],
                    op0=OP.mult, op1=OP.max)
                ps3 = mlpps.tile([1, 512], f32, tag="mlpp", space="PSUM")
                nc.tensor.matmul(out=ps3[:], lhsT=w3T[:], rhs=l2[:],
                                 start=True, stop=True)
                nc.scalar.activation(out=osb[:, cs], in_=ps3[:],
                                     func=AF.Identity, bias=b3[:], scale=1.0)
            nc.sync.dma_start(out_d[:], osb[:])

    nc.finalize()
    return nc


def host_weights(w_k1, g_k1, b_k1, m_k1, v_k1, w_k2, g_k2, b_k2, m_k2, v_k2,
                 w1, g1, b1, m1, v1, w2, g2, b2, m2, v2, w3, b3):
    f = np.float32
    s1 = (g_k1 / np.sqrt(v_k1 + f(EPS))).astype(f)
    t1 = (b_k1 - m_k1 * s1).astype(f)
    wn = w_k1[:, :C]
    wc = w_k1[:, C:]
    wnT = np.ascontiguousarray((wn * s1[:, None]).T.astype(f))
    wcnT = np.ascontiguousarray(((wc - wn) * s1[:, None]).T.astype(f))
    s2 = (g_k2 / np.sqrt(v_k2 + f(EPS))).astype(f)
    t2 = (b_k2 - m_k2 * s2).astype(f)
    w2T = np.ascontiguousarray((w_k2 * s2[:, None]).T.astype(f))
    sm1 = (g1 / np.sqrt(v1 + f(EPS))).astype(f)
    tm1 = (b1 - m1 * sm1).astype(f)
    w1s = (w1 * sm1[:, None]).astype(f)          # (256, 64)
    w1aT = np.ascontiguousarray(w1s[:128].T)      # (64, 128)
    w1bT = np.ascontiguousarray(w1s[128:].T)
    sm2 = (g2 / np.sqrt(v2 + f(EPS))).astype(f)
    tm2 = (b2 - m2 * sm2).astype(f)
    w2s = (w2 * sm2[:, None]).astype(f)          # (128, 256)
    w2maT = np.ascontiguousarray(w2s[:, :128].T)  # (128, 128)
    w2mbT = np.ascontiguousarray(w2s[:, 128:].T)
    w3T = np.ascontiguousarray(w3.T.astype(f))    # (128, 1)
    return {
        "wnT": wnT, "wcnT": wcnT, "t1": t1.reshape(C, 1),
        "w2T": w2T, "t2": t2.reshape(C, 1),
        "w1aT": w1aT, "w1bT": w1bT,
        "tm1a": tm1[:128].reshape(128, 1), "tm1b": tm1[128:].reshape(128, 1),
        "w2maT": w2maT, "w2mbT": w2mbT, "tm2": tm2.reshape(128, 1),
        "w3T": w3T, "b3": b3.reshape(1, 1).astype(f),
    }


def kernel(**inputs):
    from concourse.bass_utils import run_bass_kernel_spmd

    x = np.asarray(inputs["x"], dtype=np.float32)  # (B, C, N)
    B = x.shape[0]
    n = x.shape[2]
    w = host_weights(**{k: np.asarray(v, dtype=np.float32)
                        for k, v in inputs.items() if k != "x"})
    if n not in _cache:
        _cache[n] = build_nc(n)
    nc = _cache[n]
    in_maps = [{"x": np.ascontiguousarray(x[c]), **w} for c in range(B)]
    res = run_bass_kernel_spmd(nc, in_maps, list(range(NCORES)))
    out = np.stack([res.results[c]["out"][0] for c in range(B)], axis=0)
    return out.astype(np.float32)



# revision 8
# speedup vs baseline: 3.4386x; 3.4386x over previous
"""DGCNN prediction head on 8 Trainium2 NeuronCores.

Data-parallel over batch B=8: each core runs the full pipeline for one
sample (C=64 channels, N=4096 points, k=20 neighbors).

Per-core pipeline:
  1. Ranking key R'[i,q] = s*(2<x_i,x_q> - ||x_q||^2 + SHIFT) computed on PE
     (fp32r) with a sheared column order (slab t covers q = blk*64 + t*8 + u)
     plus an in-contraction +BIG row; a second accumulating matmul subtracts
     BIG so PSUM holds the key rounded to the 0.25 grid with its 3 low
     mantissa slots free.
  2. DVE fold packs the slab id t into those slots (key + t/32) while
     max-reducing the 8 slabs -> F1 [128,512]; exact top-24 of F1 via 3
     rounds of max8/max_index/match_replace; 5 int ops decode global
     indices q = c + (c&~7)*7 + 8t.
  3. One batched indirect DMA gathers the 20 neighbor rows of the A' table
     (A' = s1*Wn x, conv1 neighbor part).
  4. PE transposes gathered edges in pairs onto a PSUM preload of the
     center part B' = s1*(Wc-Wn)x + t1 (stacked 2x via a duplicated
     identity), so the edge add is free; Pool applies the leaky relu.
  5. EdgeConv2 as 6 fp32r matmuls (even/odd k in the stacked layout);
     scalar drains with the BN2 bias; Pool runs the max-over-k tree
     in place; DVE applies the final leaky relu into H.
  6. Point MLP 64->256->128->1 with BN folded; biases added during PSUM
     drains; leaky relu on DVE.
"""

import numpy as np

C = 64
K = 20
NEG = 0.2
EPS = 1e-5
NCORES = 8
N_FULL = 4096
NEG_FILL = -3.0e38

PACK_S = 1280.0          # ranking key scale
SHIFT = 171.0            # centers R+SHIFT in [-400, 400]
BIG = 3145728.0          # 1.5 * 2^21: rounds key to the 0.25 grid
JSTEP = 0.03125          # 1/32: slab id step packed below the grid
PAIRWISE_F32R = True

_cache = {}


def build_nc(n):
    from contextlib import ExitStack

    import concourse.bass as bass
    import concourse.bacc as bacc
    import concourse.mybir as mybir
    import concourse.tile as tile
    from concourse.masks import make_identity

    f32 = mybir.dt.float32
    f32r = mybir.dt.float32r
    bf16 = mybir.dt.bfloat16
    i32 = mybir.dt.int32
    u32 = mybir.dt.uint32
    AF = mybir.ActivationFunctionType
    OP = mybir.AluOpType

    def r(ap):
        return ap.bitcast(f32r) if PAIRWISE_F32R else ap

    def rw(ap):
        # weights/features matmuls always fp32r (feature noise is harmless)
        return ap.bitcast(f32r)

    nblk = n // 128
    nchk = n // 512

    nc = bacc.Bacc("TRN2", target_bir_lowering=False, debug=False,
                   num_devices=NCORES)

    x_d = nc.dram_tensor("x", [C, n], f32, kind="ExternalInput")
    wnT_d = nc.dram_tensor("wnT", [C, C], f32, kind="ExternalInput")
    wcnT_d = nc.dram_tensor("wcnT", [C, C], f32, kind="ExternalInput")
    t1_d = nc.dram_tensor("t1", [C, 1], f32, kind="ExternalInput")
    w2T_d = nc.dram_tensor("w2T", [C, C], f32, kind="ExternalInput")
    t2_d = nc.dram_tensor("t2", [C, 1], f32, kind="ExternalInput")
    w1aT_d = nc.dram_tensor("w1aT", [C, 128], f32, kind="ExternalInput")
    w1bT_d = nc.dram_tensor("w1bT", [C, 128], f32, kind="ExternalInput")
    tm1a_d = nc.dram_tensor("tm1a", [128, 1], f32, kind="ExternalInput")
    tm1b_d = nc.dram_tensor("tm1b", [128, 1], f32, kind="ExternalInput")
    w2maT_d = nc.dram_tensor("w2maT", [128, 128], f32, kind="ExternalInput")
    w2mbT_d = nc.dram_tensor("w2mbT", [128, 128], f32, kind="ExternalInput")
    tm2_d = nc.dram_tensor("tm2", [128, 1], f32, kind="ExternalInput")
    w3T_d = nc.dram_tensor("w3T", [128, 1], f32, kind="ExternalInput")
    b3_d = nc.dram_tensor("b3", [1, 1], f32, kind="ExternalInput")
    out_d = nc.dram_tensor("out", [1, n], f32, kind="ExternalOutput")

    with tile.TileContext(nc) as tc, ExitStack() as top:
        cpool = top.enter_context(tc.tile_pool(name="consts", bufs=1))
        dpool = top.enter_context(tc.tile_pool(name="dram", bufs=1, space="DRAM"))
        xpool = top.enter_context(tc.tile_pool(name="xaug", bufs=1))
        hpool = top.enter_context(tc.tile_pool(name="hout", bufs=1))

        # --- constants / weights ---
        ident = cpool.tile([128, 128], f32, tag="ident")
        make_identity(nc, ident[:])
        ident2 = cpool.tile([C, 128], f32, tag="ident2")
        nc.scalar.copy(out=ident2[:, :C], in_=ident[:C, :C])
        nc.scalar.copy(out=ident2[:, C:], in_=ident[:C, :C])
        ones64 = cpool.tile([C, 1], f32, tag="ones64")
        nc.vector.memset(ones64[:], 1.0)
        ones512 = cpool.tile([1, 512], f32, tag="ones512")
        nc.vector.memset(ones512[:], 1.0)
        negbig = cpool.tile([1, 128], f32, tag="negbig")
        nc.vector.memset(negbig[:], -BIG)

        def load_const(dram, shape, tag):
            t = cpool.tile(shape, f32, tag=tag)
            nc.sync.dma_start(t[:], dram[:])
            return t

        wnT = load_const(wnT_d, [C, C], "wnT")
        wcnT = load_const(wcnT_d, [C, C], "wcnT")
        t1 = load_const(t1_d, [C, 1], "t1")
        t2 = load_const(t2_d, [C, 1], "t2")
        w1aT = load_const(w1aT_d, [C, 128], "w1aT")
        w1bT = load_const(w1bT_d, [C, 128], "w1bT")
        tm1a = load_const(tm1a_d, [128, 1], "tm1a")
        tm1b = load_const(tm1b_d, [128, 1], "tm1b")
        w2maT = load_const(w2maT_d, [128, 128], "w2maT")
        w2mbT = load_const(w2mbT_d, [128, 128], "w2mbT")
        tm2 = load_const(tm2_d, [128, 1], "tm2")
        w3T = load_const(w3T_d, [128, 1], "w3T")
        b3 = load_const(b3_d, [1, 1], "b3")
        w2Ts = cpool.tile([128, C], f32, tag="w2Ts")   # w2T stacked twice
        nc.sync.dma_start(w2Ts[:C, :], w2T_d[:])
        nc.sync.dma_start(w2Ts[C:, :], w2T_d[:])

        At = dpool.tile([n, C], f32, tag="At")           # A' table, row-major
        # xaug rows: 0:64 x | 64 norm-hi(bf16) | 65 norm-res | 66 ones
        xaug = xpool.tile([C + 3, n], f32, tag="xaug")
        # x2aug rows: 0:64 2s*x | 64,65 -s | 66 BIG + s*SHIFT
        x2aug = xpool.tile([C + 3, n], f32, tag="x2aug")
        Bcm = xpool.tile([C, n], f32, tag="Bcm")         # B' channel-major
        H = hpool.tile([C, n], f32, tag="H")
        osb = hpool.tile([1, n], f32, tag="osb")

        # ---------------- stage 0: tables ----------------
        nhi_st = xpool.tile([1, n], f32, tag="nhi_st")
        nres_st = xpool.tile([1, n], f32, tag="nres_st")
        with tc.tile_pool(name="s0sb", bufs=2) as s0sb, \
             tc.tile_pool(name="s0ps", bufs=2, space="PSUM") as s0ps:
            nc.sync.dma_start(xaug[:C, :], x_d[:])
            nc.scalar.activation(out=x2aug[:C, :], in_=xaug[:C, :],
                                 func=AF.Copy, scale=2.0 * PACK_S)
            nc.vector.memset(x2aug[C:C + 2, :], -PACK_S)
            bigrow = s0sb.tile([1, 512], f32, tag="bigrow")
            nc.vector.memset(bigrow[:], BIG + PACK_S * SHIFT)
            nc.sync.dma_start(
                x2aug[C + 2:C + 3, :].rearrange("p (g c) -> p g c", g=8),
                bigrow[:].rearrange("p (g c) -> p g c", g=1)
                .to_broadcast([1, 8, 512]))
            nc.sync.dma_start(
                xaug[C + 2:C + 3, :].rearrange("p (g c) -> p g c", g=8),
                ones512[:].rearrange("p (g c) -> p g c", g=1)
                .to_broadcast([1, 8, 512]))
            for ch in range(nchk):
                cs = slice(512 * ch, 512 * (ch + 1))
                xsq = s0sb.tile([C, 512], f32, tag="xsq")
                nc.scalar.activation(out=xsq[:], in_=xaug[:C, cs], func=AF.Square)
                psxx = s0ps.tile([1, 512], f32, tag="s0px", space="PSUM")
                nc.tensor.matmul(out=psxx[:], lhsT=ones64[:], rhs=xsq[:],
                                 start=True, stop=True)
                nh16 = s0sb.tile([1, 512], bf16, tag="nh16")
                nc.scalar.copy(out=nh16[:], in_=psxx[:])
                nc.scalar.copy(out=nhi_st[:, cs], in_=nh16[:])
                nc.vector.tensor_tensor(out=nres_st[:, cs],
                                        in0=psxx[:], in1=nhi_st[:, cs],
                                        op=OP.subtract)
            nc.sync.dma_start(xaug[C:C + 1, :], nhi_st[:])
            nc.sync.dma_start(xaug[C + 1:C + 2, :], nres_st[:])
            for ch in range(nchk):
                cs = slice(512 * ch, 512 * (ch + 1))
                psa = s0ps.tile([C, 512], f32, tag="s0p", space="PSUM")
                nc.tensor.matmul(out=psa[:], lhsT=rw(wnT[:]),
                                 rhs=rw(xaug[:C, cs]), start=True, stop=True)
                ap = s0sb.tile([C, 512], f32, tag="ap")
                nc.scalar.copy(out=ap[:], in_=psa[:])
                psb = s0ps.tile([C, 512], f32, tag="s0p", space="PSUM")
                nc.tensor.matmul(out=psb[:], lhsT=rw(wcnT[:]),
                                 rhs=rw(xaug[:C, cs]), start=True, stop=True)
                nc.scalar.activation(out=Bcm[:, cs], in_=psb[:],
                                     func=AF.Identity, bias=t1[:], scale=1.0)
                pta = s0ps.tile([128, 256], f32, tag="s0t", space="PSUM")
                for j in range(4):
                    nc.tensor.transpose(out=pta[:, 64 * j:64 * (j + 1)],
                                        in_=ap[:, 128 * j:128 * (j + 1)],
                                        identity=ident[:C, :C])
                ast = s0sb.tile([128, 256], f32, tag="ast")
                nc.scalar.copy(out=ast[:], in_=pta[:])
                nc.sync.dma_start(
                    At[512 * ch:512 * (ch + 1), :].rearrange(
                        "(g p) c -> p g c", g=4),
                    ast[:].rearrange("p (g c) -> p g c", g=4))

        # sheared rhs view: slab t covers columns q = blk*64 + t*8 + u
        xaug_sh = xaug[:].rearrange("p (b t u) -> p b t u", t=8, u=8)

        # ---------------- stage 1: blocks ----------------
        with tc.tile_pool(name="fpool", bufs=2) as fpool, \
             tc.tile_pool(name="vpool", bufs=2) as vpool, \
             tc.tile_pool(name="gpool", bufs=2) as gpool, \
             tc.tile_pool(name="epool", bufs=2) as epool, \
             tc.tile_pool(name="dtree", bufs=2) as dtree, \
             tc.tile_pool(name="psR", bufs=3, space="PSUM") as psR, \
             tc.tile_pool(name="psT", bufs=2, space="PSUM") as psT, \
             tc.tile_pool(name="psE", bufs=2, space="PSUM") as psE:

            f_tiles = {}

            def emit_rank(b):
                """Pairwise keys + packed fold -> F1 [128, 512]."""
                bs = slice(128 * b, 128 * (b + 1))
                F1 = fpool.tile([128, 512], f32, tag="F1")
                ps_prev = None
                for t in range(8):
                    ps = psR.tile([128, 512], f32, tag="psr", space="PSUM")
                    nc.tensor.matmul(out=ps[:], lhsT=r(x2aug[:, bs]),
                                     rhs=r(xaug_sh[:, :, t, :]),
                                     start=True, stop=False)
                    nc.tensor.matmul(out=ps[:], lhsT=r(negbig[:]),
                                     rhs=r(ones512[:]),
                                     start=False, stop=True)
                    if t == 0:
                        ps_prev = ps
                    elif t == 1:
                        nc.vector.scalar_tensor_tensor(
                            out=F1[:], in0=ps[:], scalar=JSTEP,
                            in1=ps_prev[:], op0=OP.add, op1=OP.max)
                    else:
                        nc.vector.scalar_tensor_tensor(
                            out=F1[:], in0=ps[:], scalar=t * JSTEP,
                            in1=F1[:], op0=OP.add, op1=OP.max)
                f_tiles[b] = F1

            def emit_topk(b):
                """top-24 of F1, decode indices, start the gather."""
                F1 = f_tiles.pop(b)
                V = vpool.tile([128, 24], f32, tag="V")
                CI = vpool.tile([128, 24], u32, tag="CI")
                for rnd in range(3):
                    vs = slice(8 * rnd, 8 * (rnd + 1))
                    nc.vector.max(out=V[:, vs], in_=F1[:])
                    nc.vector.max_index(out=CI[:, vs], in_max=V[:, vs],
                                        in_values=F1[:])
                    if rnd < 2:
                        nc.vector.match_replace(out=F1[:], in_to_replace=V[:, vs],
                                                in_values=F1[:],
                                                imm_value=NEG_FILL)
                # q = c + (c & ~7)*7 + 8*t   with t = (32*v) & 7
                Wi = vpool.tile([128, 24], i32, tag="Wi")
                d1 = vpool.tile([128, 24], i32, tag="d1")
                d2 = vpool.tile([128, 24], i32, tag="d2")
                t3 = vpool.tile([128, 24], i32, tag="t3")
                GI = vpool.tile([128, 24], u32, tag="GI")
                nc.vector.tensor_scalar(out=Wi[:], in0=V[:], scalar1=32.0,
                                        scalar2=None, op0=OP.mult)
                nc.vector.tensor_scalar(out=d1[:], in0=CI[:].bitcast(i32),
                                        scalar1=-8, scalar2=7,
                                        op0=OP.bitwise_and, op1=OP.mult)
                nc.vector.tensor_scalar(out=d2[:], in0=Wi[:],
                                        scalar1=7, scalar2=8,
                                        op0=OP.bitwise_and, op1=OP.mult)
                nc.vector.tensor_tensor(out=t3[:], in0=CI[:].bitcast(i32),
                                        in1=d1[:], op=OP.add)
                nc.vector.tensor_tensor(out=GI[:].bitcast(i32), in0=t3[:],
                                        in1=d2[:], op=OP.add)
                G = gpool.tile([128, K * C], f32, tag="G")
                nc.gpsimd.indirect_dma_start(
                    out=G[:], out_offset=None,
                    in_=At[:],
                    in_offset=bass.IndirectOffsetOnAxis(ap=GI[:, :K], axis=0))
                return G

            def emit_edges(b, G):
                bs = slice(128 * b, 128 * (b + 1))
                E = epool.tile([128, K * 128 // 2], f32, tag="E")  # [128, 1280]
                bb = Bcm[:, bs].rearrange("p (k c) -> p k c", k=1)
                # waves of transpose pairs onto a B' preload
                for w, npair in ((0, 4), (1, 4), (2, 2)):
                    wcols = 128 * npair
                    pst = psT.tile([128, 512], f32, tag="pst", space="PSUM")
                    nc.tensor.matmul(out=pst[:, :wcols], lhsT=rw(ident2[:]),
                                     rhs=rw(bb.to_broadcast([C, npair, 128])),
                                     start=True, stop=False,
                                     skip_group_check=True)
                    for i in range(npair):
                        p = 4 * w + i
                        nc.tensor.matmul(
                            out=pst[:, 128 * i:128 * (i + 1)].bitcast(f32r),
                            lhsT=G[:, 128 * p:128 * (p + 1)].bitcast(f32r),
                            rhs=ident[:].bitcast(f32r),
                            is_transpose=True, start=False,
                            stop=(i == npair - 1), skip_group_check=True)
                    es = slice(512 * w, 512 * w + wcols)
                    nc.scalar.copy(out=E[:, es], in_=pst[:, :wcols])
                    nc.gpsimd.scalar_tensor_tensor(
                        out=E[:, es], in0=E[:, es], scalar=NEG,
                        in1=E[:, es], op0=OP.mult, op1=OP.max)
                # conv2: even k on rows 0:64, odd k on rows 64:128
                D = dtree.tile([C, 2560], f32, tag="D")
                for half, rows in ((0, slice(0, C)), (1, slice(C, 128))):
                    for seg, (c0, c1) in enumerate(((0, 512), (512, 1024),
                                                    (1024, 1280))):
                        pe = psE.tile([C, 512], f32, tag="pse", space="PSUM")
                        seglen = c1 - c0
                        nc.tensor.matmul(out=pe[:, :seglen],
                                         lhsT=rw(w2Ts[rows, :]),
                                         rhs=rw(E[rows, c0:c1]),
                                         start=True, stop=True)
                        dcol = 1280 * half + c0
                        nc.scalar.activation(out=D[:, dcol:dcol + seglen],
                                             in_=pe[:, :seglen],
                                             func=AF.Identity, bias=t2[:],
                                             scale=1.0)
                # max over k: in-place tree on Pool, lrelu on DVE -> H
                nc.gpsimd.tensor_tensor(out=D[:, :1280], in0=D[:, :1280],
                                        in1=D[:, 1280:], op=OP.max)
                nc.gpsimd.tensor_tensor(out=D[:, :640], in0=D[:, :640],
                                        in1=D[:, 640:1280], op=OP.max)
                nc.gpsimd.tensor_tensor(out=D[:, :256], in0=D[:, :256],
                                        in1=D[:, 256:512], op=OP.max)
                nc.gpsimd.tensor_tensor(out=D[:, :128], in0=D[:, :128],
                                        in1=D[:, 128:256], op=OP.max)
                nc.gpsimd.tensor_tensor(out=D[:, :128], in0=D[:, :128],
                                        in1=D[:, 512:640], op=OP.max)
                nc.vector.scalar_tensor_tensor(
                    out=H[:, bs], in0=D[:, :128], scalar=NEG,
                    in1=D[:, :128], op0=OP.mult, op1=OP.max)

            emit_rank(0)
            for b in range(nblk):
                G = emit_topk(b)
                if b + 1 < nblk:
                    emit_rank(b + 1)
                emit_edges(b, G)

        # ---------------- stage 2: point MLP ----------------
        with tc.tile_pool(name="mlpsb", bufs=2) as mlpsb, \
             tc.tile_pool(name="mlpps", bufs=4, space="PSUM") as mlpps:
            for ch in range(nchk):
                cs = slice(512 * ch, 512 * (ch + 1))
                l1a = mlpsb.tile([128, 512], f32, tag="l1a")
                l1b = mlpsb.tile([128, 512], f32, tag="l1b")
                ps1a = mlpps.tile([128, 512], f32, tag="mlpp", space="PSUM")
                nc.tensor.matmul(out=ps1a[:], lhsT=rw(w1aT[:]),
                                 rhs=rw(H[:, cs]), start=True, stop=True)
                nc.scalar.activation(out=l1a[:], in_=ps1a[:],
                                     func=AF.Identity, bias=tm1a[:], scale=1.0)
                nc.vector.scalar_tensor_tensor(
                    out=l1a[:], in0=l1a[:], scalar=NEG, in1=l1a[:],
                    op0=OP.mult, op1=OP.max)
                ps1b = mlpps.tile([128, 512], f32, tag="mlpp", space="PSUM")
                nc.tensor.matmul(out=ps1b[:], lhsT=rw(w1bT[:]),
                                 rhs=rw(H[:, cs]), start=True, stop=True)
                nc.scalar.activation(out=l1b[:], in_=ps1b[:],
                                     func=AF.Identity, bias=tm1b[:], scale=1.0)
                nc.vector.scalar_tensor_tensor(
                    out=l1b[:], in0=l1b[:], scalar=NEG, in1=l1b[:],
                    op0=OP.mult, op1=OP.max)
                ps2 = mlpps.tile([128, 512], f32, tag="mlpp", space="PSUM")
                nc.tensor.matmul(out=ps2[:], lhsT=rw(w2maT[:]), rhs=rw(l1a[:]),
                                 start=True, stop=False)
                nc.tensor.matmul(out=ps2[:], lhsT=rw(w2mbT[:]), rhs=rw(l1b[:]),
                                 start=False, stop=True)
                l2 = mlpsb.tile([128, 512], f32, tag="l2")
                nc.scalar.activation(out=l2[:], in_=ps2[:],
                                     func=AF.Identity, bias=tm2[:], scale=1.0)
                nc.vector.scalar_tensor_tensor(
                    out=l2[:], in0=l2[:], scalar=NEG, in1=l2[:],
                    op0=OP.mult, op1=OP.max)
                ps3 = mlpps.tile([1, 512], f32, tag="mlpp3", space="PSUM")
                nc.tensor.matmul(out=ps3[:], lhsT=rw(w3T[:]), rhs=rw(l2[:]),
                                 start=True, stop=True)
                nc.scalar.activation(out=osb[:, cs], in_=ps3[:],
                                     func=AF.Identity, bias=b3[:], scale=1.0)
            nc.sync.dma_start(out_d[:], osb[:])

    nc.finalize()
    return nc


def host_weights(w_k1, g_k1, b_k1, m_k1, v_k1, w_k2, g_k2, b_k2, m_k2, v_k2,
                 w1, g1, b1, m1, v1, w2, g2, b2, m2, v2, w3, b3):
    f = np.float32
    s1 = (g_k1 / np.sqrt(v_k1 + f(EPS))).astype(f)
    t1 = (b_k1 - m_k1 * s1).astype(f)
    wn = w_k1[:, :C]
    wc = w_k1[:, C:]
    wnT = np.ascontiguousarray((wn * s1[:, None]).T.astype(f))
    wcnT = np.ascontiguousarray(((wc - wn) * s1[:, None]).T.astype(f))
    s2 = (g_k2 / np.sqrt(v_k2 + f(EPS))).astype(f)
    t2 = (b_k2 - m_k2 * s2).astype(f)
    w2T = np.ascontiguousarray((w_k2 * s2[:, None]).T.astype(f))
    sm1 = (g1 / np.sqrt(v1 + f(EPS))).astype(f)
    tm1 = (b1 - m1 * sm1).astype(f)
    w1s = (w1 * sm1[:, None]).astype(f)          # (256, 64)
    w1aT = np.ascontiguousarray(w1s[:128].T)      # (64, 128)
    w1bT = np.ascontiguousarray(w1s[128:].T)
    sm2 = (g2 / np.sqrt(v2 + f(EPS))).astype(f)
    tm2 = (b2 - m2 * sm2).astype(f)
    w2s = (w2 * sm2[:, None]).astype(f)          # (128, 256)
    w2maT = np.ascontiguousarray(w2s[:, :128].T)  # (128, 128)
    w2mbT = np.ascontiguousarray(w2s[:, 128:].T)
    w3T = np.ascontiguousarray(w3.T.astype(f))    # (128, 1)
    return {
        "wnT": wnT, "wcnT": wcnT, "t1": t1.reshape(C, 1),
        "w2T": w2T, "t2": t2.reshape(C, 1),
        "w1aT": w1aT, "w1bT": w1bT,
        "tm1a": tm1[:128].reshape(128, 1), "tm1b": tm1[128:].reshape(128, 1),
        "w2maT": w2maT, "w2mbT": w2mbT, "tm2": tm2.reshape(128, 1),
        "w3T": w3T, "b3": b3.reshape(1, 1).astype(f),
    }


def kernel(**inputs):
    from concourse.bass_utils import run_bass_kernel_spmd

    x = np.asarray(inputs["x"], dtype=np.float32)  # (B, C, N)
    B = x.shape[0]
    n = x.shape[2]
    w = host_weights(**{k: np.asarray(v, dtype=np.float32)
                        for k, v in inputs.items() if k != "x"})
    if n not in _cache:
        _cache[n] = build_nc(n)
    nc = _cache[n]
    in_maps = [{"x": np.ascontiguousarray(x[c]), **w} for c in range(B)]
    res = run_bass_kernel_spmd(nc, in_maps, list(range(NCORES)))
    out = np.stack([res.results[c]["out"][0] for c in range(B)], axis=0)
    return out.astype(np.float32)
